# revision 7
# baseline (speedup 1.0000x reference)
"""Causal multi-head attention (B=4, S=2048, D=1024, H=16, hd=64) on 8 TRN2
NeuronCores.

Sharding: core c = (batch b = c//2, head-group g = c%2). Each core computes
QKV projections for its 8 heads (Megatron column-split), causal attention,
and a partial out-projection (row-split); the host sums the two head-group
partials per batch and adds the bias.

On-device layout (bf16 compute, fp32 PSUM accumulation):
  xT  [1024, 2048]  x[b]^T           (din on partitions)
  qT/kT as [d_g, S] transposed tiles: head-pair t -> partitions
        [0:64] head 2t, [64:128] head 2t+1
  v   [k-tile 128, 8 heads, 65]: col 64 is ones (sumexp lands in the ctx^T
        psum row 64 for free during the attn*V matmul)
  scores^T psum tiles [k 128, 2 heads, q 512] (2 banks): head pair packed
        via PE row tiling (K=64 each), one exp / one mask-mul over both
  attn = exp(scores/8), no max-subtraction (|s|/8 <= ~3), causal handled by
        skipping k-tiles above the diagonal, restricting the q-range on
        diagonal tiles (s0 = dd*128), and a mask multiply for the boundary

Schedule: the attention inner loop is ACT(exp)-paced (~1.17us/k-tile vs
~0.64us of PE work), so all projection / out-projection matmuls are queued
as "fill" units and spliced into the PE instruction stream at matmul
granularity between the attention matmuls, driven by a simulated PE/ACT
clock. Scores are emitted one k-tile ahead of the attn*V consumers
(software pipelining against the in-order PE queue), out-projection of
q-chunk s overlaps the attention of q-chunk s+1, and V/QK projections are
staged so each phase's inputs are ready just in time.
"""

from collections import deque

import numpy as np
import ml_dtypes

import concourse.bass as bass
import concourse.tile as tile
from concourse import bacc, mybir
from concourse.bass_utils import run_bass_kernel_spmd

P = 128          # partitions
S = 2048         # sequence length (one batch per core)
DIN = 1024       # model dim
DG = 512         # head-group width per core (8 heads x 64)
HD = 64          # head dim
NH = 8           # heads per core
QC = 512         # q-chunk (matmul free dim)
NQC = S // QC    # 4 q-chunks
NKT = S // P     # 16 k-tiles
KDT = DIN // P   # 8 din k-tiles
NHP = 4          # head pairs per core
F32 = mybir.dt.float32
BF16 = mybir.dt.bfloat16
EXP = mybir.ActivationFunctionType.Exp

MM = 512 / 2400.0      # us, one N=512 matmul issue slot (warm PE)
SEM = 0.1              # us, semaphore handoff margin

_CACHE = {}


def _emit(tc, d):
    nc = tc.nc
    with (
        nc.allow_low_precision(reason="bf16 attention pipeline"),
        tc.tile_pool(name="persist", bufs=1) as pp,
        tc.tile_pool(name="work", bufs=4) as wp,
        tc.tile_pool(name="psc", bufs=2, space="PSUM") as psc,
        tc.tile_pool(name="ppj", bufs=2, space="PSUM") as ppj,
        tc.tile_pool(name="pcx", bufs=1, space="PSUM") as pcx,
    ):
        # ---- persistent SBUF tiles ----
        xT = [pp.tile([P, S], BF16, tag=f"xT{k}", name=f"xT{k}") for k in range(KDT)]
        wq = [pp.tile([P, DG], BF16, tag=f"wq{k}", name=f"wq{k}") for k in range(KDT)]
        wk = [pp.tile([P, DG], BF16, tag=f"wk{k}", name=f"wk{k}") for k in range(KDT)]
        wv = [pp.tile([P, DG], BF16, tag=f"wv{k}", name=f"wv{k}") for k in range(KDT)]
        wo = [pp.tile([P, DIN], BF16, tag=f"wo{k}", name=f"wo{k}") for k in range(4)]
        qT = [pp.tile([P, S], BF16, tag=f"qT{t}", name=f"qT{t}") for t in range(NHP)]
        kT = [pp.tile([P, S], BF16, tag=f"kT{t}", name=f"kT{t}") for t in range(NHP)]
        vv = [pp.tile([P, NH, HD + 1], BF16, tag=f"v{m}", name=f"v{m}") for m in range(NKT)]
        cx = [pp.tile([P, S], BF16, tag=f"cx{t}", name=f"cx{t}") for t in range(NHP)]
        msk = pp.tile([P, 4, 2, QC], BF16, tag="msk", name="msk")

        # ---- input DMAs, priority order, spread across idle engine queues.
        # Gate for the first matmuls: wv + xT[:, 0:512]; then wq/wk (first
        # q/k chains), then the rest of x, then wo/masks (needed later). ----
        qs = [nc.sync, nc.scalar, nc.gpsimd]
        for k in range(KDT):
            qs[k % 3].dma_start(wv[k][:], d["wvT"][k * P:(k + 1) * P, :])
            qs[(k + 1) % 3].dma_start(xT[k][:, 0:QC], d["xT"][k * P:(k + 1) * P, 0:QC])
        for k in range(KDT):
            qs[k % 3].dma_start(wq[k][:], d["wqT"][k * P:(k + 1) * P, :])
            qs[(k + 1) % 3].dma_start(wk[k][:], d["wkT"][k * P:(k + 1) * P, :])
        for dd in range(4):
            for h in range(2):
                nc.scalar.dma_start(
                    msk[:, dd, h, :], d["masks"][:, dd * QC:(dd + 1) * QC]
                )
        for k in range(KDT):
            qs[k % 3].dma_start(
                xT[k][:, QC:2 * QC], d["xT"][k * P:(k + 1) * P, QC:2 * QC]
            )
        for k in range(KDT):
            qs[k % 3].dma_start(
                xT[k][:, 2 * QC:S], d["xT"][k * P:(k + 1) * P, 2 * QC:S]
            )
        for k in range(4):
            nc.scalar.dma_start(wo[k][:], d["woT"][k * P:(k + 1) * P, :])

        # ---- emission scheduler state: simulated engine clocks (us) ----
        st = {"pe": 7.0, "act": 7.0}   # start after the input-DMA gate
        fills = deque()                # (label, pe_cost_us, emit_fn)
        left = {}                      # label -> entries still queued
        done = set()

        def _q(label, entries):
            left[label] = len(entries)
            for cost, fn in entries:
                fills.append((label, cost, fn))

        def _pop():
            label, cost, fn = fills.popleft()
            fn()
            st["pe"] += cost
            left[label] -= 1
            if left[label] == 0:
                done.add(label)

        def drain(target):
            while fills and st["pe"] + fills[0][1] <= target:
                _pop()

        def need(label):
            if label not in left:
                raise RuntimeError(f"fill {label} never queued")
            while label not in done:
                _pop()

        # ---- fill-unit builders ----
        def v_chain(m):
            ps = ppj.tile([P, QC], F32, tag="pj", name="ps")
            ent = []
            for k in range(KDT):
                def mm(k=k, ps=ps):
                    nc.tensor.matmul(
                        ps[:], xT[k][:, m * P:(m + 1) * P], wv[k][:],
                        start=(k == 0), stop=(k == KDT - 1),
                    )
                ent.append((MM, mm))

            def post(ps=ps):
                nc.vector.tensor_copy(
                    vv[m][:, :, 0:HD], ps[:].rearrange("p (h e) -> p h e", h=NH)
                )
                nc.vector.memset(vv[m][:, :, HD:HD + 1], 1.0)
            ent.append((0.02, post))
            return ent

        def qk_chain(t, w, s):
            wt, dst = ((wq, qT), (wk, kT))[w]
            ps = ppj.tile([P, QC], F32, tag="pj", name="ps")
            ent = []
            for k in range(KDT):
                def mm(k=k, ps=ps):
                    nc.tensor.matmul(
                        ps[:], wt[k][:, t * P:(t + 1) * P],
                        xT[k][:, s * QC:(s + 1) * QC],
                        start=(k == 0), stop=(k == KDT - 1),
                    )
                ent.append((MM, mm))

            def post(ps=ps):
                nc.vector.tensor_copy(dst[t][:, s * QC:(s + 1) * QC], ps[:])
            ent.append((0.02, post))
            return ent

        def o_chain(s, o):
            ps = ppj.tile([P, QC], F32, tag="pj", name="ps")
            ent = []
            for k in range(4):
                def mm(k=k, ps=ps):
                    nc.tensor.matmul(
                        ps[:], wo[k][:, o * P:(o + 1) * P],
                        cx[k][:, s * QC:(s + 1) * QC],
                        start=(k == 0), stop=(k == 3),
                    )
                ent.append((MM, mm))

            def post(ps=ps):
                ob = wp.tile([P, QC], F32, tag="ob", name="ob")
                nc.vector.tensor_copy(ob[:], ps[:])
                nc.sync.dma_start(
                    d["outT"][o * P:(o + 1) * P, s * QC:(s + 1) * QC], ob[:]
                )
            ent.append((0.02, post))
            return ent

        # ---- attention chunk (hp, sc), scores one k-tile ahead of attn*V ----
        def attn_chunk(hp, sc):
            nkt = 4 * (sc + 1)
            cps = pcx.tile([HD + 1, 2, QC], F32, tag="cx", name="cps")
            exp_done = {}
            a_t = {}

            def attn_v(j, last):
                a, s0 = a_t.pop(j)
                need(("v", j))
                drain(exp_done[j])
                st["pe"] = max(st["pe"], exp_done[j])
                nc.tensor.matmul(
                    cps[:, 0, s0:], vv[j][:, 2 * hp, :], a[:, 0, s0:],
                    start=(j == 0), stop=last,
                )
                nc.tensor.matmul(
                    cps[:, 1, s0:], vv[j][:, 2 * hp + 1, :], a[:, 1, s0:],
                    start=(j == 0), stop=last,
                )
                st["pe"] += 2 * (QC - s0) / 2400.0

            for j in range(nkt):
                dd = j - 4 * sc
                s0 = max(dd, 0) * P
                if j >= 2:
                    # psum double-buffer: scores j reuse exp j-2's bank
                    drain(exp_done[j - 2])
                    st["pe"] = max(st["pe"], exp_done[j - 2])
                sps = psc.tile([P, 2, QC], F32, tag="sc", name="sps")
                nc.tensor.matmul(
                    sps[:, 0, s0:],
                    kT[hp][0:HD, j * P:(j + 1) * P],
                    qT[hp][0:HD, sc * QC + s0:(sc + 1) * QC],
                    start=True, stop=True,
                )
                nc.tensor.matmul(
                    sps[:, 1, s0:],
                    kT[hp][HD:P, j * P:(j + 1) * P],
                    qT[hp][HD:P, sc * QC + s0:(sc + 1) * QC],
                    start=True, stop=True,
                )
                st["pe"] += (QC - s0) / 2400.0 + 0.005
                a = wp.tile([P, 2, QC], BF16, tag="a", name="a", bufs=6)
                nc.scalar.activation(a[:, :, s0:], sps[:, :, s0:], EXP, scale=0.125)
                st["act"] = max(st["act"], st["pe"] + SEM) \
                    + (2 * (QC - s0) + 352) / 1200.0
                ed = st["act"] + SEM
                if dd >= 0:
                    nc.vector.tensor_mul(
                        a[:, :, s0:s0 + P], a[:, :, s0:s0 + P],
                        msk[:, dd, :, s0:s0 + P],
                    )
                    ed += 0.2
                exp_done[j] = ed
                a_t[j] = (a, s0)
                if j >= 1:
                    attn_v(j - 1, last=False)
            attn_v(nkt - 1, last=True)

            # normalize: rows 0:64 are ctx^T, row 64 is sumexp; reshape the
            # sumexp row to [128, 8] via SBUF DMA so the DVE iterative
            # reciprocal runs on free-dim 8, broadcast back across partitions
            cb = wp.tile([HD + 1, 2, QC], F32, tag="cb", name="cb", bufs=2)
            nc.vector.tensor_copy(cb[:], cps[:])
            zt = wp.tile([P, 8], F32, tag="zt", name="zt")
            nc.sync.dma_start(zt[:], cb[HD:HD + 1, :, :])
            rt = wp.tile([P, 8], F32, tag="rt", name="rt")
            nc.vector.reciprocal(rt[:], zt[:])
            rc = wp.tile([P, 2, QC], F32, tag="rc", name="rc")
            nc.sync.dma_start(rc[0:1, :, :], rt[:])
            bs = wp.tile([HD, 2, QC], F32, tag="bs", name="bs", bufs=2)
            nc.gpsimd.partition_broadcast(bs[:], rc[0:1, :, :])
            nc.vector.tensor_mul(
                cx[hp][0:HD, sc * QC:(sc + 1) * QC], cb[0:HD, 0, :], bs[:, 0, :]
            )
            cxs = wp.tile([HD, QC], BF16, tag="cxs", name="cxs")
            nc.vector.tensor_mul(cxs[:], cb[0:HD, 1, :], bs[:, 1, :])
            nc.sync.dma_start(cx[hp][HD:P, sc * QC:(sc + 1) * QC], cxs[:])
            st["pe"] += 0.3   # next chunk's attn*V waits on the cb copy (WAR)

        # ---- queue prologue + phase fills, run the chunk schedule ----
        _q(("q", 0, 0), qk_chain(0, 0, 0))
        _q(("k", 0, 0), qk_chain(0, 1, 0))
        for m in range(4):
            _q(("v", m), v_chain(m))
        need(("q", 0, 0))
        need(("k", 0, 0))

        for sc in range(NQC):
            # fills that become runnable / needed during this phase
            if sc == 0:
                for t in (1, 2, 3):
                    _q(("q", t, 0), qk_chain(t, 0, 0))
                    _q(("k", t, 0), qk_chain(t, 1, 0))
            if sc < NQC - 1:
                for m in range(4 * (sc + 1), 4 * (sc + 2)):
                    _q(("v", m), v_chain(m))
                for t in range(NHP):
                    _q(("q", t, sc + 1), qk_chain(t, 0, sc + 1))
                    _q(("k", t, sc + 1), qk_chain(t, 1, sc + 1))
            if sc >= 1:
                for o in range(DIN // P):
                    _q(("o", sc - 1, o), o_chain(sc - 1, o))
            for hp in range(NHP):
                need(("q", hp, sc))
                for c in range(sc + 1):
                    need(("k", hp, c))
                attn_chunk(hp, sc)

        # leftover fills (late out-projections), then the final q-chunk's
        # out-projection
        while fills:
            _pop()
        for o in range(DIN // P):
            for cost, fn in o_chain(NQC - 1, o):
                fn()


def _build():
    if "nc" in _CACHE:
        return _CACHE["nc"]
    nc = bacc.Bacc("TRN2", target_bir_lowering=False, debug=False, num_devices=8)
    d = {
        "xT": nc.dram_tensor("xT", [DIN, S], BF16, kind="ExternalInput").ap(),
        "wqT": nc.dram_tensor("wqT", [DIN, DG], BF16, kind="ExternalInput").ap(),
        "wkT": nc.dram_tensor("wkT", [DIN, DG], BF16, kind="ExternalInput").ap(),
        "wvT": nc.dram_tensor("wvT", [DIN, DG], BF16, kind="ExternalInput").ap(),
        "woT": nc.dram_tensor("woT", [DG, DIN], BF16, kind="ExternalInput").ap(),
        "masks": nc.dram_tensor("masks", [P, 4 * QC], BF16, kind="ExternalInput").ap(),
        "outT": nc.dram_tensor("outT", [DIN, S], F32, kind="ExternalOutput").ap(),
    }
    with tile.TileContext(nc) as tc:
        _emit(tc, d)
    nc.compile()
    _CACHE["nc"] = nc
    return nc


def _masks_np():
    r = np.arange(P)[:, None]
    j = np.arange(QC)[None, :]
    return np.concatenate(
        [(j >= r + dd * P).astype(ml_dtypes.bfloat16) for dd in range(4)], axis=1
    )


def kernel(x, Wq, Wk, Wv, Wo, bo, _run_kwargs=None, _return_res=False):
    x = np.asarray(x)
    Wq, Wk, Wv, Wo, bo = (np.asarray(a) for a in (Wq, Wk, Wv, Wo, bo))
    B = x.shape[0]
    nc = _build()

    def b16(a):
        return np.ascontiguousarray(a).astype(ml_dtypes.bfloat16)

    masks = _masks_np()
    in_maps = []
    for c in range(8):
        b, g = divmod(c, 2)
        in_maps.append({
            "xT": b16(x[b].T),
            "wqT": b16(Wq[g * DG:(g + 1) * DG, :].T),
            "wkT": b16(Wk[g * DG:(g + 1) * DG, :].T),
            "wvT": b16(Wv[g * DG:(g + 1) * DG, :].T),
            "woT": b16(Wo[:, g * DG:(g + 1) * DG].T),
            "masks": masks,
        })

    res = run_bass_kernel_spmd(nc, in_maps, list(range(8)), **(_run_kwargs or {}))
    out = np.empty((B, S, DIN), np.float32)
    for b in range(B):
        p = res.results[2 * b]["outT"] + res.results[2 * b + 1]["outT"]
        out[b] = p.T + bo.astype(np.float32)
    if _return_res:
        return out, res
    return out


# revision 12
# speedup vs baseline: 1.0120x; 1.0120x over previous
"""Causal multi-head attention (B=4, S=2048, D=1024, H=16, hd=64) on 8 TRN2
NeuronCores.

Sharding: core c = (batch b = c//2, head-group g = c%2). Each core computes
QKV projections for its 8 heads (Megatron column-split), causal attention,
and a partial out-projection (row-split); the host sums the two head-group
partials per batch and adds the bias.

On-device layout (bf16 compute, fp32 PSUM accumulation):
  xT  [1024, 2048]  x[b]^T           (din on partitions)
  qT/kT as [d_g, S] transposed tiles: head-pair t -> partitions
        [0:64] head 2t, [64:128] head 2t+1
  v   [k-tile 128, 8 heads, 65]: col 64 is ones (sumexp lands in the ctx^T
        psum row 64 for free during the attn*V matmul)
  scores^T psum tiles [k 128, 2 heads, q 512] (2 banks): head pair packed
        via PE row tiling (K=64 each), one exp / one mask-mul over both
  attn = exp(scores/8), no max-subtraction (|s|/8 <= ~3), causal handled by
        skipping k-tiles above the diagonal, restricting the q-range on
        diagonal tiles (s0 = dd*128), and a mask multiply for the boundary

Schedule: the attention inner loop is ACT(exp)-paced (~1.17us/k-tile vs
~0.64us of PE work), so all projection / out-projection matmuls are queued
as "fill" units and spliced into the PE instruction stream at matmul
granularity between the attention matmuls, driven by a simulated PE/ACT
clock. Scores are emitted one k-tile ahead of the attn*V consumers
(software pipelining against the in-order PE queue), out-projection of
q-chunk s overlaps the attention of q-chunk s+1, and V/QK projections are
staged so each phase's inputs are ready just in time.
"""

from collections import deque

import numpy as np
import ml_dtypes

import concourse.bass as bass
import concourse.tile as tile
from concourse import bacc, mybir
from concourse.bass_utils import run_bass_kernel_spmd

P = 128          # partitions
S = 2048         # sequence length (one batch per core)
DIN = 1024       # model dim
DG = 512         # head-group width per core (8 heads x 64)
HD = 64          # head dim
NH = 8           # heads per core
QC = 512         # q-chunk (matmul free dim)
NQC = S // QC    # 4 q-chunks
NKT = S // P     # 16 k-tiles
KDT = DIN // P   # 8 din k-tiles
NHP = 4          # head pairs per core
F32 = mybir.dt.float32
BF16 = mybir.dt.bfloat16
EXP = mybir.ActivationFunctionType.Exp

MM = 512 / 2400.0      # us, one N=512 matmul issue slot (warm PE)
SEM = 0.15             # us, semaphore handoff margin
BONUS = 0.3            # us, deliberate over-fill before each attn*V so it
                       # never reaches the head of the in-order PE queue
                       # before its exp is done (a wait exposes ~170ns of
                       # pipeline drain)

_CACHE = {}


def _emit(tc, d):
    nc = tc.nc
    with (
        nc.allow_low_precision(reason="bf16 attention pipeline"),
        tc.tile_pool(name="persist", bufs=1) as pp,
        tc.tile_pool(name="work", bufs=4) as wp,
        tc.tile_pool(name="psc", bufs=2, space="PSUM") as psc,
        tc.tile_pool(name="ppj", bufs=2, space="PSUM") as ppj,
        tc.tile_pool(name="pcx", bufs=1, space="PSUM") as pcx,
    ):
        # ---- persistent SBUF tiles ----
        xT = [pp.tile([P, S], BF16, tag=f"xT{k}", name=f"xT{k}") for k in range(KDT)]
        wq = [pp.tile([P, DG], BF16, tag=f"wq{k}", name=f"wq{k}") for k in range(KDT)]
        wk = [pp.tile([P, DG], BF16, tag=f"wk{k}", name=f"wk{k}") for k in range(KDT)]
        wv = [pp.tile([P, DG], BF16, tag=f"wv{k}", name=f"wv{k}") for k in range(KDT)]
        wo = [pp.tile([P, DIN], BF16, tag=f"wo{k}", name=f"wo{k}") for k in range(4)]
        qT = [pp.tile([P, S], BF16, tag=f"qT{t}", name=f"qT{t}") for t in range(NHP)]
        kT = [pp.tile([P, S], BF16, tag=f"kT{t}", name=f"kT{t}") for t in range(NHP)]
        vv = [pp.tile([P, NH, HD + 1], BF16, tag=f"v{m}", name=f"v{m}") for m in range(NKT)]
        cx = [pp.tile([P, S], BF16, tag=f"cx{t}", name=f"cx{t}") for t in range(NHP)]
        msk = pp.tile([P, 4, 2, QC], BF16, tag="msk", name="msk")

        # ---- input DMAs, priority order, spread across idle engine queues.
        # Gate for the first matmuls: wv + xT[:, 0:512]; then wq/wk (first
        # q/k chains), then the rest of x, then wo/masks (needed later). ----
        qs = [nc.sync, nc.scalar, nc.gpsimd]
        for k in range(KDT):
            qs[k % 3].dma_start(wv[k][:], d["wvT"][k * P:(k + 1) * P, :])
            qs[(k + 1) % 3].dma_start(xT[k][:, 0:QC], d["xT"][k * P:(k + 1) * P, 0:QC])
        for k in range(KDT):
            qs[k % 3].dma_start(wq[k][:], d["wqT"][k * P:(k + 1) * P, :])
            qs[(k + 1) % 3].dma_start(wk[k][:], d["wkT"][k * P:(k + 1) * P, :])
        for dd in range(4):
            for h in range(2):
                nc.scalar.dma_start(
                    msk[:, dd, h, :], d["masks"][:, dd * QC:(dd + 1) * QC]
                )
        for k in range(KDT):
            qs[k % 3].dma_start(
                xT[k][:, QC:2 * QC], d["xT"][k * P:(k + 1) * P, QC:2 * QC]
            )
        for k in range(KDT):
            qs[k % 3].dma_start(
                xT[k][:, 2 * QC:S], d["xT"][k * P:(k + 1) * P, 2 * QC:S]
            )
        for k in range(4):
            nc.scalar.dma_start(wo[k][:], d["woT"][k * P:(k + 1) * P, :])

        # ---- emission scheduler state: simulated engine clocks (us) ----
        st = {"pe": 7.0, "act": 7.0}   # start after the input-DMA gate
        fills = deque()                # (label, pe_cost_us, emit_fn)
        left = {}                      # label -> entries still queued
        done = set()

        def _q(label, entries):
            left[label] = len(entries)
            for cost, fn in entries:
                fills.append((label, cost, fn))

        def _pop():
            label, cost, fn = fills.popleft()
            fn()
            st["pe"] += cost
            left[label] -= 1
            if left[label] == 0:
                done.add(label)

        def drain(target):
            while fills and st["pe"] + fills[0][1] <= target:
                _pop()

        def need(label):
            if label not in left:
                raise RuntimeError(f"fill {label} never queued")
            while label not in done:
                _pop()

        # ---- fill-unit builders ----
        def v_chain(m):
            ps = ppj.tile([P, QC], F32, tag="pj", name="ps")
            ent = []
            for k in range(KDT):
                def mm(k=k, ps=ps):
                    nc.tensor.matmul(
                        ps[:], xT[k][:, m * P:(m + 1) * P], wv[k][:],
                        start=(k == 0), stop=(k == KDT - 1),
                    )
                ent.append((MM, mm))

            def post(ps=ps):
                nc.vector.tensor_copy(
                    vv[m][:, :, 0:HD], ps[:].rearrange("p (h e) -> p h e", h=NH)
                )
                nc.vector.memset(vv[m][:, :, HD:HD + 1], 1.0)
            ent.append((0.02, post))
            return ent

        def qk_chain(t, w, s):
            wt, dst = ((wq, qT), (wk, kT))[w]
            ps = ppj.tile([P, QC], F32, tag="pj", name="ps")
            ent = []
            for k in range(KDT):
                def mm(k=k, ps=ps):
                    nc.tensor.matmul(
                        ps[:], wt[k][:, t * P:(t + 1) * P],
                        xT[k][:, s * QC:(s + 1) * QC],
                        start=(k == 0), stop=(k == KDT - 1),
                    )
                ent.append((MM, mm))

            def post(ps=ps):
                nc.vector.tensor_copy(dst[t][:, s * QC:(s + 1) * QC], ps[:])
            ent.append((0.02, post))
            return ent

        def o_chain(s, o):
            ps = ppj.tile([P, QC], F32, tag="pj", name="ps")
            ent = []
            for k in range(4):
                def mm(k=k, ps=ps):
                    nc.tensor.matmul(
                        ps[:], wo[k][:, o * P:(o + 1) * P],
                        cx[k][:, s * QC:(s + 1) * QC],
                        start=(k == 0), stop=(k == 3),
                    )
                ent.append((MM, mm))

            def post(ps=ps):
                ob = wp.tile([P, QC], F32, tag="ob", name="ob")
                nc.vector.tensor_copy(ob[:], ps[:])
                nc.sync.dma_start(
                    d["outT"][o * P:(o + 1) * P, s * QC:(s + 1) * QC], ob[:]
                )
            ent.append((0.02, post))
            return ent

        # ---- attention chunk (hp, sc), scores one k-tile ahead of attn*V ----
        def attn_chunk(hp, sc):
            nkt = 4 * (sc + 1)
            cps = pcx.tile([HD + 1, 2, QC], F32, tag="cx", name="cps")
            exp_done = {}
            a_t = {}

            def attn_v(j, last):
                a, s0 = a_t.pop(j)
                need(("v", j))
                drain(exp_done[j] + BONUS)
                st["pe"] = max(st["pe"], exp_done[j])
                nc.tensor.matmul(
                    cps[:, 0, s0:], vv[j][:, 2 * hp, :], a[:, 0, s0:],
                    start=(j == 0), stop=last,
                )
                nc.tensor.matmul(
                    cps[:, 1, s0:], vv[j][:, 2 * hp + 1, :], a[:, 1, s0:],
                    start=(j == 0), stop=last,
                )
                st["pe"] += 2 * (QC - s0) / 2400.0

            for j in range(nkt):
                dd = j - 4 * sc
                s0 = max(dd, 0) * P
                if j >= 2:
                    # psum double-buffer: scores j reuse exp j-2's bank
                    drain(exp_done[j - 2])
                    st["pe"] = max(st["pe"], exp_done[j - 2])
                sps = psc.tile([P, 2, QC], F32, tag="sc", name="sps")
                nc.tensor.matmul(
                    sps[:, 0, s0:],
                    kT[hp][0:HD, j * P:(j + 1) * P],
                    qT[hp][0:HD, sc * QC + s0:(sc + 1) * QC],
                    start=True, stop=True,
                )
                nc.tensor.matmul(
                    sps[:, 1, s0:],
                    kT[hp][HD:P, j * P:(j + 1) * P],
                    qT[hp][HD:P, sc * QC + s0:(sc + 1) * QC],
                    start=True, stop=True,
                )
                st["pe"] += (QC - s0) / 2400.0 + 0.005
                a = wp.tile([P, 2, QC], BF16, tag="a", name="a", bufs=6)
                nc.scalar.activation(a[:, :, s0:], sps[:, :, s0:], EXP, scale=0.125)
                st["act"] = max(st["act"], st["pe"] + SEM) \
                    + (2 * (QC - s0) + 352) / 1200.0
                ed = st["act"] + SEM
                if dd >= 0:
                    nc.vector.tensor_mul(
                        a[:, :, s0:s0 + P], a[:, :, s0:s0 + P],
                        msk[:, dd, :, s0:s0 + P],
                    )
                    ed += 0.35
                exp_done[j] = ed
                a_t[j] = (a, s0)
                if j >= 1:
                    attn_v(j - 1, last=False)
            attn_v(nkt - 1, last=True)

            # normalize: rows 0:64 are ctx^T, row 64 is sumexp; reshape the
            # sumexp row to [128, 8] via SBUF DMA so the DVE iterative
            # reciprocal runs on free-dim 8, broadcast back across partitions
            cb = wp.tile([HD + 1, 2, QC], F32, tag="cb", name="cb", bufs=2)
            nc.vector.tensor_copy(cb[:], cps[:])
            zt = wp.tile([P, 8], F32, tag="zt", name="zt")
            nc.sync.dma_start(zt[:], cb[HD:HD + 1, :, :])
            rt = wp.tile([P, 8], F32, tag="rt", name="rt")
            nc.vector.reciprocal(rt[:], zt[:])
            rc = wp.tile([P, 2, QC], F32, tag="rc", name="rc")
            nc.sync.dma_start(rc[0:1, :, :], rt[:])
            bs = wp.tile([HD, 2, QC], F32, tag="bs", name="bs", bufs=2)
            nc.gpsimd.partition_broadcast(bs[:], rc[0:1, :, :])
            nc.vector.tensor_mul(
                cx[hp][0:HD, sc * QC:(sc + 1) * QC], cb[0:HD, 0, :], bs[:, 0, :]
            )
            cxs = wp.tile([HD, QC], BF16, tag="cxs", name="cxs")
            nc.vector.tensor_mul(cxs[:], cb[0:HD, 1, :], bs[:, 1, :])
            nc.sync.dma_start(cx[hp][HD:P, sc * QC:(sc + 1) * QC], cxs[:])
            st["pe"] += 0.3   # next chunk's attn*V waits on the cb copy (WAR)

        # ---- PE warm-up: ~4.5us of dummy matmuls during the input-DMA gate
        # so the HAM clock gate reaches 8/8 before the first real chain (the
        # first ~3.4us of sustained PE activity run at 1.2 GHz otherwise) ----
        wrm = pp.tile([P, P], BF16, tag="wrm", name="wrm")
        nc.vector.memset(wrm[:], 0.125)
        wps = ppj.tile([P, QC], F32, tag="pj", name="wps")
        for i in range(80):
            nc.tensor.matmul(
                wps[0:HD, (i % 2) * HD:(i % 2) * HD + HD],
                wrm[:, 0:HD], wrm[:, HD:P],
                start=True, stop=True,
            )

        # ---- queue prologue + phase fills, run the chunk schedule ----
        _q(("q", 0, 0), qk_chain(0, 0, 0))
        _q(("k", 0, 0), qk_chain(0, 1, 0))
        for m in range(4):
            _q(("v", m), v_chain(m))
        need(("q", 0, 0))
        need(("k", 0, 0))

        for sc in range(NQC):
            # fills that become runnable / needed during this phase
            if sc == 0:
                for t in (1, 2, 3):
                    _q(("q", t, 0), qk_chain(t, 0, 0))
                    _q(("k", t, 0), qk_chain(t, 1, 0))
            if sc < NQC - 1:
                for m in range(4 * (sc + 1), 4 * (sc + 2)):
                    _q(("v", m), v_chain(m))
                for t in range(NHP):
                    _q(("q", t, sc + 1), qk_chain(t, 0, sc + 1))
                    _q(("k", t, sc + 1), qk_chain(t, 1, sc + 1))
            if sc >= 1:
                for o in range(DIN // P):
                    _q(("o", sc - 1, o), o_chain(sc - 1, o))
            for hp in range(NHP):
                need(("q", hp, sc))
                for c in range(sc + 1):
                    need(("k", hp, c))
                attn_chunk(hp, sc)

        # leftover fills (late out-projections), then the final q-chunk's
        # out-projection
        while fills:
            _pop()
        for o in range(DIN // P):
            for cost, fn in o_chain(NQC - 1, o):
                fn()


def _build():
    if "nc" in _CACHE:
        return _CACHE["nc"]
    nc = bacc.Bacc("TRN2", target_bir_lowering=False, debug=False, num_devices=8)
    d = {
        "xT": nc.dram_tensor("xT", [DIN, S], BF16, kind="ExternalInput").ap(),
        "wqT": nc.dram_tensor("wqT", [DIN, DG], BF16, kind="ExternalInput").ap(),
        "wkT": nc.dram_tensor("wkT", [DIN, DG], BF16, kind="ExternalInput").ap(),
        "wvT": nc.dram_tensor("wvT", [DIN, DG], BF16, kind="ExternalInput").ap(),
        "woT": nc.dram_tensor("woT", [DG, DIN], BF16, kind="ExternalInput").ap(),
        "masks": nc.dram_tensor("masks", [P, 4 * QC], BF16, kind="ExternalInput").ap(),
        "outT": nc.dram_tensor("outT", [DIN, S], F32, kind="ExternalOutput").ap(),
    }
    with tile.TileContext(nc) as tc:
        _emit(tc, d)
    nc.compile()
    _CACHE["nc"] = nc
    return nc


def _masks_np():
    r = np.arange(P)[:, None]
    j = np.arange(QC)[None, :]
    return np.concatenate(
        [(j >= r + dd * P).astype(ml_dtypes.bfloat16) for dd in range(4)], axis=1
    )


def kernel(x, Wq, Wk, Wv, Wo, bo, _run_kwargs=None, _return_res=False):
    x = np.asarray(x)
    Wq, Wk, Wv, Wo, bo = (np.asarray(a) for a in (Wq, Wk, Wv, Wo, bo))
    B = x.shape[0]
    nc = _build()

    def b16(a):
        return np.ascontiguousarray(a).astype(ml_dtypes.bfloat16)

    masks = _masks_np()
    in_maps = []
    for c in range(8):
        b, g = divmod(c, 2)
        in_maps.append({
            "xT": b16(x[b].T),
            "wqT": b16(Wq[g * DG:(g + 1) * DG, :].T),
            "wkT": b16(Wk[g * DG:(g + 1) * DG, :].T),
            "wvT": b16(Wv[g * DG:(g + 1) * DG, :].T),
            "woT": b16(Wo[:, g * DG:(g + 1) * DG].T),
            "masks": masks,
        })

    res = run_bass_kernel_spmd(nc, in_maps, list(range(8)), **(_run_kwargs or {}))
    out = np.empty((B, S, DIN), np.float32)
    for b in range(B):
        p = res.results[2 * b]["outT"] + res.results[2 * b + 1]["outT"]
        out[b] = p.T + bo.astype(np.float32)
    if _return_res:
        return out, res
    return out
